# revision 18
# baseline (speedup 1.0000x reference)
"""Trainium2 Bass kernel for nn_MultiHeadAttention_89232240541956.

Computes, for B=8, S=4096, H=1024, ATTN=1024, EXT=1152:
    x_ext = [h | broadcast(g) | l]                       [B, S, 1152]
    q = relu(x_ext @ Wq + bq); k = relu(x_ext @ Wk + bk) [B, S, 1024]
    scores = sum(q * k, -1) / 32, masked to -1e9 where mask == 1

Sharding: data-parallel over batch — core b owns batch b.

Key transformations:
  - v (Wv, bv) is dead code in the reference's early-return path — skipped.
  - g @ Wq[1024:1088] is constant over seq for a batch — folded into the
    bias on the host, so the device contracts over 1024 (h) + 64 (l) only.
  - Bias folded into the matmul as one extra contraction row (ones-row in
    x^T against a bias-row in W).
  - Host pre-transposes to x^T so the contraction dim lands on SBUF
    partitions with no on-device transposes.
  - h-contraction runs in fp8 E4M3 with DoubleRow (2 MACs/cell/cycle);
    x is pre-scaled by 16 and W by 64 (lossless exponent shifts) to stay
    in E4M3's normal range. l+bias chunk stays bf16. The combined 2^10
    scale and the 1/sqrt(1024) are divided out in the epilogue.
  - Epilogue: ScalarE relu (PSUM->SBUF bf16), DVE multiply, ScalarE
    Copy-with-accum for the row reduction; masking on host at unshard.
"""

import numpy as np
import ml_dtypes

B, S, H, LOC = 8, 4096, 1024, 64
ATTN = 1024
NE = 8                    # bf16 path: full 128-row h chunks
NC8 = 4                   # fp8 path: paired h chunks (256 contraction rows)
KL = LOC + 1              # 65 rows: l | ones/bias
SBLK = 512                # seq columns per DMA block
NBLK = S // SBLK          # 8
NT = SBLK // 128          # 4 seq tiles (128 tokens) per block
NCOL = S // 128           # 32 output columns

BF16 = ml_dtypes.bfloat16

USE_FP8 = False
XSCALE = 16.0
WSCALE = 64.0

_CACHE = {}


def _build_nc(use_fp8=USE_FP8):
    import concourse.bass as bass
    import concourse.mybir as mybir
    import concourse.tile as tile
    from concourse import bacc

    dt = mybir.dt
    nc = bacc.Bacc(None, target_bir_lowering=False)
    if use_fp8:
        # host layout: [c, blk, p, j, s] rows flattened; row pair j on the
        # same partition p covers contraction rows c*256 + 2p + j.
        xh_d = nc.dram_tensor(
            "xh", [NC8 * NBLK * 128, 2 * SBLK], dt.float8e4, kind="ExternalInput"
        )
        wq_d = nc.dram_tensor(
            "wq", [NC8 * 128, 2 * ATTN], dt.float8e4, kind="ExternalInput"
        )
        wk_d = nc.dram_tensor(
            "wk", [NC8 * 128, 2 * ATTN], dt.float8e4, kind="ExternalInput"
        )
    else:
        xh_d = nc.dram_tensor("xh", [H, S], dt.bfloat16, kind="ExternalInput")
        wq_d = nc.dram_tensor("wq", [H, ATTN], dt.bfloat16, kind="ExternalInput")
        wk_d = nc.dram_tensor("wk", [H, ATTN], dt.bfloat16, kind="ExternalInput")
    xl_d = nc.dram_tensor("xl", [KL, S], dt.bfloat16, kind="ExternalInput")
    wql_d = nc.dram_tensor("wql", [KL, ATTN], dt.bfloat16, kind="ExternalInput")
    wkl_d = nc.dram_tensor("wkl", [KL, ATTN], dt.bfloat16, kind="ExternalInput")
    out = nc.dram_tensor("out", [128, NCOL], dt.float32, kind="ExternalOutput")

    scale = 1.0 / 32.0
    if use_fp8:
        scale /= (XSCALE * WSCALE) ** 2

    with tile.TileContext(nc) as tc:
        with (
            tc.tile_pool(name="wpool", bufs=1) as wpool,
            tc.tile_pool(name="xpool", bufs=2) as xpool,
            tc.tile_pool(name="epool", bufs=2) as epool,
            tc.tile_pool(name="opool", bufs=1) as opool,
            tc.tile_pool(name="psum", bufs=1, space="PSUM") as psum,
        ):
            if use_fp8:
                wq_sb = wpool.tile([128, NC8, 2, ATTN], dt.float8e4, tag="wq")
                wk_sb = wpool.tile([128, NC8, 2, ATTN], dt.float8e4, tag="wk")
                xh0 = xpool.tile([128, NC8, 2, SBLK], dt.float8e4, tag="xh")
                nch = NC8
            else:
                wq_sb = wpool.tile([128, NE, ATTN], dt.bfloat16, tag="wq")
                wk_sb = wpool.tile([128, NE, ATTN], dt.bfloat16, tag="wk")
                xh0 = xpool.tile([128, NE, SBLK], dt.bfloat16, tag="xh")
                nch = NE

            def dma_xh(xh, blk):
                if use_fp8:
                    for c in range(NC8):
                        r0 = (c * NBLK + blk) * 128
                        nc.sync.dma_start(xh[:, c, :, :], xh_d[r0 : r0 + 128, :])
                else:
                    c0 = blk * SBLK
                    for e in range(NE):
                        nc.sync.dma_start(
                            xh[:, e, :], xh_d[e * 128 : (e + 1) * 128, c0 : c0 + SBLK]
                        )

            # Pre-warm the PE clock (HAM) with dummy matmuls on a zeroed tile
            # while the startup DMAs are in flight — no data dependencies, so
            # they run right after the NEFF preamble and the real matmul
            # stream starts at the full 2.4 GHz.
            warm_src = wpool.tile([128, 512], dt.bfloat16, tag="warm")
            nc.gpsimd.memset(warm_src[:], 0.0)
            warm_ps = psum.tile(
                [128, ATTN], dt.float32, tag="psq", bufs=2, name="warm_ps"
            )
            for _ in range(20):
                nc.tensor.matmul(
                    warm_ps[:, 0:512], warm_src[:, 0:128], warm_src[:],
                    start=True, stop=True,
                )

            # startup: interleave block-0 x chunks with the weight chunks so
            # the first tile's accumulation group is ready ASAP. Chunk 0's
            # weights are DMA'd in halves so the very first matmul only waits
            # for ~384KB.
            wql_sb = wpool.tile([KL, ATTN], dt.bfloat16, tag="wql")
            wkl_sb = wpool.tile([KL, ATTN], dt.bfloat16, tag="wkl")
            for c in range(nch):
                if use_fp8:
                    nc.sync.dma_start(wq_sb[:, c, :, :], wq_d[c * 128 : (c + 1) * 128, :])
                    nc.sync.dma_start(wk_sb[:, c, :, :], wk_d[c * 128 : (c + 1) * 128, :])
                    r0 = c * NBLK * 128
                    nc.sync.dma_start(xh0[:, c, :, :], xh_d[r0 : r0 + 128, :])
                else:
                    if c == 0:
                        nc.sync.dma_start(xh0[:, 0, :], xh_d[0:128, 0:SBLK])
                        for nh in range(2):
                            n0 = nh * 512
                            nc.sync.dma_start(
                                wq_sb[:, 0, n0 : n0 + 512], wq_d[0:128, n0 : n0 + 512]
                            )
                            nc.sync.dma_start(
                                wk_sb[:, 0, n0 : n0 + 512], wk_d[0:128, n0 : n0 + 512]
                            )
                    else:
                        nc.sync.dma_start(wq_sb[:, c, :], wq_d[c * 128 : (c + 1) * 128, :])
                        nc.sync.dma_start(wk_sb[:, c, :], wk_d[c * 128 : (c + 1) * 128, :])
                        nc.sync.dma_start(xh0[:, c, :], xh_d[c * 128 : (c + 1) * 128, 0:SBLK])
            nc.sync.dma_start(wql_sb[:], wql_d[:])
            nc.sync.dma_start(wkl_sb[:], wkl_d[:])
            xl0 = xpool.tile([KL, SBLK], dt.bfloat16, tag="xl")
            nc.sync.dma_start(xl0[:], xl_d[:, 0:SBLK])

            score_sb = opool.tile([128, NCOL], dt.float32, tag="score")

            pm = mybir.MatmulPerfMode.DoubleRow if use_fp8 else None

            def lhs_of(xh, c, s0):
                if use_fp8:
                    return xh[:, c, :, s0 : s0 + 128]
                return xh[:, c, s0 : s0 + 128]

            def rhs_of(w_sb, c, n0):
                if use_fp8:
                    return w_sb[:, c, :, n0 : n0 + 512]
                return w_sb[:, c, n0 : n0 + 512]

            def epilogue(psq, psk, col, split=False):
                if split:
                    # Final tile: process halves with a DVE reduction so the
                    # post-last-matmul chain is as short as possible. The
                    # 1/32 (and fp8 descale) rides on the q relu.
                    sc2 = epool.tile([128, 2], dt.float32, tag="sc2")
                    for nh in range(2):
                        n0 = nh * 512
                        qsh = epool.tile([128, 512], dt.bfloat16, tag="qsh")
                        nc.scalar.activation(
                            qsh[:], psq[:, n0 : n0 + 512],
                            mybir.ActivationFunctionType.Relu, scale=scale,
                        )
                        ksh = epool.tile([128, 512], dt.bfloat16, tag="ksh")
                        nc.scalar.activation(
                            ksh[:], psk[:, n0 : n0 + 512],
                            mybir.ActivationFunctionType.Relu,
                        )
                        prh = epool.tile([128, 512], dt.bfloat16, tag="prh")
                        nc.vector.tensor_mul(prh[:], qsh[:], ksh[:])
                        nc.vector.tensor_reduce(
                            sc2[:, nh : nh + 1], prh[:],
                            axis=mybir.AxisListType.X, op=mybir.AluOpType.add,
                        )
                    nc.vector.tensor_reduce(
                        score_sb[:, col : col + 1], sc2[:],
                        axis=mybir.AxisListType.X, op=mybir.AluOpType.add,
                    )
                    return
                qsb = epool.tile([128, ATTN], dt.bfloat16, tag="qsb")
                nc.scalar.activation(
                    qsb[:], psq[:], mybir.ActivationFunctionType.Relu
                )
                ksb = epool.tile([128, ATTN], dt.bfloat16, tag="ksb")
                nc.scalar.activation(
                    ksb[:], psk[:], mybir.ActivationFunctionType.Relu
                )
                prod = epool.tile([128, ATTN], dt.bfloat16, tag="prod")
                nc.vector.tensor_mul(prod[:], qsb[:], ksb[:])
                cpy = epool.tile([128, ATTN], dt.bfloat16, tag="cpy")
                nc.scalar.activation(
                    cpy[:],
                    prod[:],
                    mybir.ActivationFunctionType.Copy,
                    scale=scale,
                    accum_out=score_sb[:, col : col + 1],
                )

            for blk in range(NBLK):
                c0 = blk * SBLK
                if blk == 0:
                    xh = xh0
                    xl = xl0
                else:
                    if use_fp8:
                        xh = xpool.tile([128, NC8, 2, SBLK], dt.float8e4, tag="xh")
                    else:
                        xh = xpool.tile([128, NE, SBLK], dt.bfloat16, tag="xh")
                    dma_xh(xh, blk)
                    xl = xpool.tile([KL, SBLK], dt.bfloat16, tag="xl")
                    nc.sync.dma_start(xl[:], xl_d[:, c0 : c0 + SBLK])

                for t in range(NT):
                    psq = psum.tile(
                        [128, ATTN], dt.float32, tag="psq", bufs=2,
                        name=f"psq_{blk}_{t}",
                    )
                    psk = psum.tile(
                        [128, ATTN], dt.float32, tag="psk", bufs=2,
                        name=f"psk_{blk}_{t}",
                    )
                    for c in range(nch):
                        lhs = lhs_of(xh, c, t * 128)
                        for nh in range(2):
                            n0 = nh * 512
                            nc.tensor.matmul(
                                psq[:, n0 : n0 + 512], lhs, rhs_of(wq_sb, c, n0),
                                start=(c == 0), stop=False, perf_mode=pm,
                            )
                            nc.tensor.matmul(
                                psk[:, n0 : n0 + 512], lhs, rhs_of(wk_sb, c, n0),
                                start=(c == 0), stop=False, perf_mode=pm,
                            )
                    lhs_l = xl[:, t * 128 : (t + 1) * 128]
                    for nh in range(2):
                        n0 = nh * 512
                        nc.tensor.matmul(
                            psq[:, n0 : n0 + 512], lhs_l, wql_sb[:, n0 : n0 + 512],
                            start=False, stop=True,
                        )
                        nc.tensor.matmul(
                            psk[:, n0 : n0 + 512], lhs_l, wkl_sb[:, n0 : n0 + 512],
                            start=False, stop=True,
                        )
                    epilogue(
                        psq, psk, blk * NT + t,
                        split=(blk == NBLK - 1 and t == NT - 1),
                    )

            nc.sync.dma_start(out[:], score_sb[:])

    nc.compile()
    return nc


def _get_nc():
    if "nc" not in _CACHE:
        _CACHE["nc"] = _build_nc()
    return _CACHE["nc"]


def prep_in_maps(h, mask, g, l, Wq, bq, Wk, bk, Wv=None, bv=None, use_fp8=USE_FP8):
    import concourse.mybir as mybir

    FP8 = mybir.dt.np(mybir.dt.float8e4)

    h = np.asarray(h, dtype=np.float32)
    g = np.asarray(g, dtype=np.float32)
    l_ = np.asarray(l, dtype=np.float32)
    Wq = np.asarray(Wq, dtype=np.float32)
    bq = np.asarray(bq, dtype=np.float32)
    Wk = np.asarray(Wk, dtype=np.float32)
    bk = np.asarray(bk, dtype=np.float32)

    # Fold the per-batch g contribution into the bias (fp32 on host).
    bq_eff = bq[None, :] + g @ Wq[H : H + LOC]          # [B, ATTN]
    bk_eff = bk[None, :] + g @ Wk[H : H + LOC]

    xs = XSCALE if use_fp8 else 1.0
    ws = WSCALE if use_fp8 else 1.0

    if use_fp8:
        # [H, ATTN] * 64 -> fp8, laid out as [c, p, j, n] row pairs.
        wq_h = np.ascontiguousarray((Wq[:H] * ws).astype(FP8)).reshape(
            NC8 * 128, 2 * ATTN
        )
        wk_h = np.ascontiguousarray((Wk[:H] * ws).astype(FP8)).reshape(
            NC8 * 128, 2 * ATTN
        )
    else:
        wq_h = (Wq[:H] * ws).astype(BF16)
        wk_h = (Wk[:H] * ws).astype(BF16)

    # l rows (scaled like x/W so the PSUM scale is uniform) + bias row.
    wql = np.empty((KL, ATTN), dtype=BF16)
    wql[:LOC] = Wq[H + LOC :] * ws
    wkl = np.empty((KL, ATTN), dtype=BF16)
    wkl[:LOC] = Wk[H + LOC :] * ws

    in_maps = []
    for b in range(B):
        if use_fp8:
            x8 = (h[b].T * xs).astype(FP8)              # [H, S]
            # [c, blk, p, j, s] -> rows (c*NBLK+blk)*128+p, cols j*SBLK+s
            xh = np.ascontiguousarray(
                x8.reshape(NC8, 128, 2, NBLK, SBLK).transpose(0, 3, 1, 2, 4)
            ).reshape(NC8 * NBLK * 128, 2 * SBLK)
        else:
            xh = np.ascontiguousarray(h[b].T).astype(BF16)
        xl = np.empty((KL, S), dtype=BF16)
        xl[:LOC] = l_[b].T * xs
        xl[LOC] = xs
        # ones-row carries xs, so the bias row needs only ws: xs*ws*b total.
        wql_b = wql.copy()
        wql_b[LOC] = bq_eff[b] * ws
        wkl_b = wkl.copy()
        wkl_b[LOC] = bk_eff[b] * ws
        in_maps.append(
            {"xh": xh, "wq": wq_h, "wk": wk_h, "xl": xl, "wql": wql_b, "wkl": wkl_b}
        )
    return in_maps


def kernel(h, mask, g, l, Wq, bq, Wk, bk, Wv=None, bv=None):
    from concourse.bass_utils import run_bass_kernel_spmd

    mask = np.asarray(mask)
    in_maps = prep_in_maps(h, mask, g, l, Wq, bq, Wk, bk)

    nc = _get_nc()
    res = run_bass_kernel_spmd(nc, in_maps, core_ids=list(range(B)), trace=False)

    scores = np.empty((B, S), dtype=np.float32)
    for b in range(B):
        scores[b] = res.results[b]["out"].T.reshape(S)
    return np.where(mask == 1, np.float32(-1e9), scores).astype(np.float32)
